# revision 57
# baseline (speedup 1.0000x reference)
"""TRN2 Bass kernel for nn_LoRACuetLinear (equivariant LoRA linear).

Math: for each irrep block j (9 blocks of 192 features; block j uses irrep
k(j) in {0,1,2}), out_seg = seg @ W_eff[k] where
  W_eff[k] = pw_base * Wb[k] + SCALING * pw_base * pw_B * (WA[k] @ WB[k])
(the LoRA branch folds exactly into the base weight since everything is
linear).

Device strategy (8 cores, data-parallel over nodes):
  - Host transposes x to x_T [features, rows] per core so the contraction
    dim (mul/feature) lies on SBUF partitions; the device then runs
    weights-stationary matmuls out_T = W^T x_T with the moving dim = rows.
  - Default mode "f16x1" (~5e-4 absmax rel, vs the 2e-2 gate): single fp16
    plane of x, single matmul pass, fp16 output.  Relative to the fp32-
    accurate "f16x3" this cuts HBM traffic 94.8 -> 44.3 MB/core and PE work
    3x (measured 351us -> ~143-147us):
      * Tile-major fully-contiguous dram layouts ([128, 13*ROWS] +
        [64, ROWS] per direction, exact-size SBUF tiles per row-tile size):
        every DMA merges into one fat run per partition.  Sliced/strided
        transfers degrade to small segments and lose ~25% DMA rate.
      * Input DMAs dispatch on the Sync queue, output DMAs on the Scalar
        (Activation) HWDGE queue: out-dispatch waits (copies done) never
        head-of-line-block input prefetch.
      * ALL psum->sbuf cast-copies on Scalar (no Vector/GpSimd split): a
        single self-ordered queue [copies_i, out_i, copies_i+1 ...].  Every
        engine-splitting variant measured slower (psum ring banks recycle
        in-order; bursty cross-engine copy assignment convoys the PE).
      * Row-tile schedule (256, 1024x5, 618, 256): small first tile so the
        PE starts ~6us earlier, small last tile so the final unoverlapped
        output drain is short, and as FEW tiles as possible -- every tile
        pays a full 32-LDWEIGHTS sweep regardless of rows (consolidating
        the tail from [512,256,106] to [618,256] measured ~5us faster).
      * Steady state is HBM-bound: 16 DMA engines ~385 GB/s sustained with
        ~15us fixed framework preamble/teardown.
  - Weights (LoRA exactly folded into the base, fp16) are packed per
    128-row output section into a block-diagonal [128, 32*128] layout so
    every matmul has M=128 at psum partition base 0.
  - Fallback modes kept for experiments: "f16x3" (fp32-accurate 3-pass),
    "f32r3" (float32r 3-pass with on-device DVE split) and "f32r1"
    (single-pass float32r, ~1e-4 rel).
"""

import sys

sys.path.insert(0, "/opt/trn_rl_repo")

import os
import numpy as np

import concourse.bass as bass
import concourse.tile as tile
from concourse import bacc, mybir
from concourse.bass_utils import run_bass_kernel_spmd
# ---- problem constants (hardcoded per contract) ----
MUL = 192
DIMS = (1, 3, 5)
RANK = 8
SCALING = 2.0
N_NODES = 50000
FEAT = MUL * sum(DIMS)  # 1728
NCORES = 8
ROWS = N_NODES // NCORES  # 6250
FPAD = 1792  # 14 * 128
NSEC = FPAD // 128  # 14
R = 352  # row-tile (moving dim); 6250 = 17*352 + 266 (all tiles >= 256)
RF16 = 512  # row-tile for the f16 path (smaller SBUF tiles allow 512)
RT1 = 1024  # row-tile for the single-pass f16 path (two 512 psum halves)
PSUM_N = 512  # max fp32 cols per psum bank
MODE = os.environ.get("LORA_KERNEL_MODE", "f16x1")  # f16x1 | f16x3 | f32r3 | f32r1
BLK_IRREP = [0] + [1] * 3 + [2] * 5

_MASK11 = np.uint32(0xFFFFF000)  # keep sign+exp+11 mantissa bits


def _section_mms():
    """Enumerate matmuls as (section, chunk, r0, r1, windex).

    Section s covers padded output rows [128s, 128s+128); chunk c covers
    padded input rows [128c, 128c+128).  (s, c) participates iff the
    block-diagonal weight has support there; r0:r1 is the nonzero input-row
    range within the chunk (always base 0 or 64, size 64 or 128).
    """
    sup = np.zeros((FPAD, FPAD), dtype=bool)
    for j in range(sum(DIMS)):
        sup[192 * j : 192 * j + 192, 192 * j : 192 * j + 192] = True
    mms = []
    wi = 0
    for s in range(NSEC):
        for c in range(NSEC):
            sl = sup[128 * c : 128 * c + 128, 128 * s : 128 * s + 128]
            nz = np.nonzero(sl.any(axis=1))[0]
            if len(nz) == 0:
                continue
            r0 = (int(nz[0]) // 64) * 64
            r1 = ((int(nz[-1]) + 64) // 64) * 64
            mms.append((s, c, r0, r1, wi))
            wi += 1
    return mms


_MMS = _section_mms()
NW = len(_MMS)  # 32 packed weight slots of [128, 128]


def _pack_weights(W_eff):
    """Build the packed per-section weight [128, NW*128] from W_eff [3,192,192]."""
    W_big = np.zeros((FPAD, FPAD), dtype=np.float32)
    for j, k in enumerate(BLK_IRREP):
        W_big[192 * j : 192 * j + 192, 192 * j : 192 * j + 192] = W_eff[k]
    wpk = np.zeros((128, NW * 128), dtype=np.float32)
    for s, c, r0, r1, wi in _MMS:
        wpk[:, wi * 128 : (wi + 1) * 128] = W_big[
            128 * c : 128 * c + 128, 128 * s : 128 * s + 128
        ]
    return wpk


def _row_tiles(r):
    tiles = []
    r0 = 0
    while r0 < ROWS:
        tiles.append((r0, min(r, ROWS - r0)))
        r0 += r
    return tiles


# f16x1 tile schedule: small first tile so the PE starts after a 0.9MB DMA
# instead of 3.7MB, small last tile so the final (unoverlappable) output DMA
# is short.  Sums to ROWS.
F16X1_SIZES = (256, 1024, 1024, 1024, 1024, 1024, 618, 256)
assert sum(F16X1_SIZES) == ROWS
NSEC_FULL = NSEC - 1  # 13 full 128-partition chunks; chunk 13 has 64 rows
# sbuf ring depth per tile size (1024 is the steady-state size; the odd
# sizes appear at the ramp/tail and never overlap themselves)
F16X1_BUFS = {1024: 3, 618: 1, 512: 1, 256: 1, 106: 1}


def _row_tiles_f16x1():
    tiles = []
    r0 = 0
    for sz in F16X1_SIZES:
        tiles.append((r0, sz))
        r0 += sz
    return tiles


def _build_nc_f16x1():
    """Single-pass fp16 kernel with fp16 output (~4.5e-4 absmax rel).

    The 2e-2 correctness gate leaves ~40x headroom over fp16 quantization
    noise, so ship one fp16 plane of x (half the input bytes of the fp32-
    accurate path), run one matmul pass (1/3 the PE work), and return the
    output as fp16 (half the output bytes).  psum->sbuf cast-copies
    alternate between the Scalar and Vector engines so neither serializes
    against the ~139us of DMA this leaves.
    """
    f32 = mybir.dt.float32
    f16 = mybir.dt.float16

    nc = bacc.Bacc("TRN2", target_bir_lowering=False, debug=False)
    # Tile-major contiguous dram layout: every transfer is one large
    # contiguous segment per partition (no 512B/2KB striding), for any tile
    # size.  Chunk 13 only has 64 real partitions (FEAT = 13.5 * 128), so it
    # lives in its own [64, ROWS] params and the zero half is never moved.
    x1a_in = nc.declare_dram_parameter(
        "x1a", [128, NSEC_FULL * ROWS], f16, isOutput=False
    )
    x1b_in = nc.declare_dram_parameter("x1b", [64, ROWS], f16, isOutput=False)
    wh_in = nc.declare_dram_parameter("wh", [128, NW * 128], f16, isOutput=False)
    ota_out = nc.declare_dram_parameter(
        "ota", [128, NSEC_FULL * ROWS], f16, isOutput=True
    )
    otb_out = nc.declare_dram_parameter("otb", [64, ROWS], f16, isOutput=True)

    sec_list = [[m for m in _MMS if m[0] == s] for s in range(NSEC)]

    with tile.TileContext(nc) as tc:
        with (
            tc.tile_pool(name="wp", bufs=1) as wp,
            tc.tile_pool(name="hp", bufs=1) as hp,
            tc.tile_pool(name="op", bufs=1) as op,
            tc.tile_pool(name="ps", bufs=8, space="PSUM") as ps,
        ):
            wh = wp.tile([128, NW * 128], f16, tag="wh")

            for wave in [[t] for t in enumerate(_row_tiles_f16x1())]:
                xhs, ots, odsts = [], [], []
                for ti, (r0, rt) in wave:
                    # exact-size tiles per rt: a fully dense destination lets
                    # the DMA merge each partition's data into one contiguous
                    # run (sliced tiles degrade to rt*2-byte segments)
                    xh = hp.tile(
                        [128, NSEC, rt], f16, tag=f"xh{rt}", name=f"xh{rt}",
                        bufs=F16X1_BUFS[rt],
                    )
                    xsrc = x1a_in.ap()[
                        :, NSEC_FULL * r0 : NSEC_FULL * (r0 + rt)
                    ].rearrange("p (c r) -> p c r", c=NSEC_FULL)
                    if ti == 0:
                        # head FIFO ordering: engine queues drain strictly in
                        # dispatch order, and the first matmul gates on
                        # (first chunks of tile 0) + (weights).  Send only
                        # chunks 0-3 ahead of wh, the rest behind it -- the
                        # early sections consume chunks in order while the
                        # remainder lands.  (Splitting wh itself was 12us
                        # WORSE: its tail queued behind tile 1's input.)
                        nc.sync.dma_start(xh[:, :4, :], xsrc[:, :4, :])
                        nc.sync.dma_start(wh[:], wh_in[:])
                        nc.sync.dma_start(
                            xh[:, 4:NSEC_FULL, :], xsrc[:, 4:NSEC_FULL, :]
                        )
                    else:
                        nc.sync.dma_start(xh[:, :NSEC_FULL, :], xsrc)
                    nc.sync.dma_start(
                        xh[0:64, NSEC_FULL, :], x1b_in.ap()[:, r0 : r0 + rt]
                    )
                    ot = op.tile(
                        [128, NSEC, rt], f16, tag=f"ot{rt}", name=f"ot{rt}",
                        bufs=min(2, F16X1_BUFS[rt]),
                    )
                    xhs.append(xh)
                    ots.append(ot)
                    odsts.append(
                        ota_out.ap()[
                            :, NSEC_FULL * r0 : NSEC_FULL * (r0 + rt)
                        ].rearrange("p (c r) -> p c r", c=NSEC_FULL)
                    )

                for s in range(NSEC):
                    # (tile, half) pairs processed weight-major so matmuls
                    # sharing a stationary slice are adjacent and walrus
                    # elides the LDWEIGHTS reloads
                    parts = [
                        (w, h0, min(PSUM_N, rt - h0))
                        for w, (ti, (r0, rt)) in enumerate(wave)
                        for h0 in range(0, rt, PSUM_N)
                    ]
                    psums = [
                        ps.tile([128, PSUM_N], f32, tag="ps", name=f"ps{pi}")
                        for pi in range(len(parts))
                    ]
                    nmm = len(sec_list[s])
                    for i, (_, c, k0, k1, wi) in enumerate(sec_list[s]):
                        for pi, (w, h0, hn) in enumerate(parts):
                            nc.tensor.matmul(
                                psums[pi][:, :hn],
                                wh[k0:k1, wi * 128 : (wi + 1) * 128],
                                xhs[w][k0:k1, c, h0 : h0 + hn],
                                start=(i == 0),
                                stop=(i == nmm - 1),
                            )
                    mp = 64 if s == NSEC - 1 else 128  # real out rows in section
                    # all copies on Scalar: one self-ordered queue (copies +
                    # out-DMA dispatches).  Measured dead ends: every
                    # Vector-copy split was 10-27us slower (DVE psum reads
                    # appear to contend with PE psum writes), and GpSimd
                    # psum-source copies fail to compile in neuronxcc.
                    for pi, (w, h0, hn) in enumerate(parts):
                        nc.scalar.copy(
                            ots[w][:mp, s, h0 : h0 + hn], psums[pi][:mp, :hn]
                        )
                # output DMAs dispatch whole-tile on the Scalar (Activation)
                # HWDGE queue so their copy-completion waits never head-of-
                # line-block the input prefetches on the Sync queue.
                # Measured dead ends: outputs on Sync (~+5us, head-of-line),
                # and mid-tile split outputs on either queue (+4..+10us --
                # the extra dispatches and partial-tile transfers cost more
                # than the stream smoothing buys).
                for w, (ti, (r0, rt)) in enumerate(wave):
                    nc.scalar.dma_start(
                        odsts[w][:, :NSEC_FULL, :], ots[w][:, :NSEC_FULL, :]
                    )
                    nc.scalar.dma_start(
                        otb_out.ap()[:, r0 : r0 + rt],
                        ots[w][0:64, NSEC_FULL, :],
                    )

    nc.finalize()
    return nc


def _build_nc(mode):
    if mode == "f16x1":
        return _build_nc_f16x1()
    fr = mybir.dt.float32r
    f32 = mybir.dt.float32
    f16 = mybir.dt.float16
    f16_mode = mode == "f16x3"
    three_pass = mode in ("f32r3", "f16x3")
    wdt = f16 if f16_mode else fr
    r_tile = RF16 if f16_mode else R

    nc = bacc.Bacc("TRN2", target_bir_lowering=False, debug=False)
    if f16_mode:
        # host pre-splits x into two fp16 planes (x = x1 + x2 to 22 bits),
        # pre-tiled as [rowtile, partition, chunk*R] so each partition's
        # per-rowtile data is one contiguous segment for the DMA
        nt = len(_row_tiles(r_tile))
        x1_in = nc.declare_dram_parameter(
            "x1", [nt, 128, NSEC * r_tile], f16, isOutput=False
        )
        x2_in = nc.declare_dram_parameter(
            "x2", [nt, 128, NSEC * r_tile], f16, isOutput=False
        )
    else:
        xdt_dram = f32 if three_pass else fr
        xt_in = nc.declare_dram_parameter("xt", [FPAD, ROWS], xdt_dram, isOutput=False)
        xt_src = xt_in.ap().rearrange("(c p) r -> p c r", p=128)
    wh_in = nc.declare_dram_parameter("wh", [128, NW * 128], wdt, isOutput=False)
    if three_pass:
        wl_in = nc.declare_dram_parameter("wl", [128, NW * 128], wdt, isOutput=False)
    ot_out = nc.declare_dram_parameter("ot", [FPAD, ROWS], f32, isOutput=True)

    ot_dst = ot_out.ap().rearrange("(c p) r -> p c r", p=128)

    sec_list = [[m for m in _MMS if m[0] == s] for s in range(NSEC)]

    xbufs = 3 if f16_mode else 2
    with tile.TileContext(nc) as tc:
        with (
            tc.tile_pool(name="wp", bufs=1) as wp,
            tc.tile_pool(name="xp", bufs=2) as xp,
            tc.tile_pool(name="hp", bufs=xbufs) as hp,
            tc.tile_pool(name="lp", bufs=xbufs) as lp,
            tc.tile_pool(name="op", bufs=2) as op,
            tc.tile_pool(name="ps", bufs=6, space="PSUM") as ps,
        ):
            wh = wp.tile([128, NW * 128], wdt, tag="wh")
            nc.sync.dma_start(wh[:], wh_in[:])
            if three_pass:
                wl = wp.tile([128, NW * 128], wdt, tag="wl")
                nc.sync.dma_start(wl[:], wl_in[:])

            for ti, (r0, rt) in enumerate(_row_tiles(r_tile)):
                if f16_mode:
                    xh = hp.tile([128, NSEC, r_tile], f16, tag="xh")
                    xl = lp.tile([128, NSEC, r_tile], f16, tag="xl")
                    nc.sync.dma_start(
                        xh[:], x1_in[ti].rearrange("p (c r) -> p c r", c=NSEC)
                    )
                    nc.sync.dma_start(
                        xl[:], x2_in[ti].rearrange("p (c r) -> p c r", c=NSEC)
                    )
                    passes = [(xh, wh), (xl, wh), (xh, wl)]
                elif three_pass:
                    # X1 = rn11(X), X2 = rn11(X - X1).  The raw X tile must be
                    # a genuine float32 memloc: walrus rounds float32r-memloc
                    # inputs on read, so an in-place split would cancel to 0.
                    # Rounding happens on the DVE cast writes.
                    x = xp.tile([128, NSEC, r_tile], f32, tag="x")
                    nc.sync.dma_start(x[:, :, :rt], xt_src[:, :, r0 : r0 + rt])
                    xh = hp.tile([128, NSEC, r_tile], wdt, tag="xh")
                    xl = lp.tile([128, NSEC, r_tile], wdt, tag="xl")
                    nc.vector.tensor_copy(xh[:, :, :rt], x[:, :, :rt])
                    nc.vector.tensor_sub(xl[:, :, :rt], x[:, :, :rt], xh[:, :, :rt])
                    passes = [(xh, wh), (xl, wh), (xh, wl)]
                else:
                    x = xp.tile([128, NSEC, r_tile], fr, tag="x")
                    nc.sync.dma_start(x[:, :, :rt], xt_src[:, :, r0 : r0 + rt])
                    passes = [(x, wh)]

                ot = op.tile([128, NSEC, r_tile], f32, tag="ot")
                for s in range(NSEC):
                    psum = ps.tile([128, r_tile], f32, tag="ps")
                    # order so matmuls sharing a stationary slice are
                    # adjacent (lets walrus ldw-opt elide reloads)
                    if len(passes) == 3:
                        (xa, wa), (xb, _), (_, wc) = passes
                        seq = [
                            (x, w, c, k0, k1, wi)
                            for _, c, k0, k1, wi in sec_list[s]
                            for x, w in ((xa, wa), (xb, wa))
                        ] + [
                            (xa, wc, c, k0, k1, wi)
                            for _, c, k0, k1, wi in sec_list[s]
                        ]
                    else:
                        seq = [
                            (x, w, c, k0, k1, wi)
                            for x, w in passes
                            for _, c, k0, k1, wi in sec_list[s]
                        ]
                    for i, (xsrc, wsrc, c, k0, k1, wi) in enumerate(seq):
                        nc.tensor.matmul(
                            psum[:, :rt],
                            wsrc[k0:k1, wi * 128 : (wi + 1) * 128],
                            xsrc[k0:k1, c, :rt],
                            start=(i == 0),
                            stop=(i == len(seq) - 1),
                        )
                    nc.scalar.copy(ot[:, s, :rt], psum[:, :rt])
                nc.sync.dma_start(ot_dst[:, :, r0 : r0 + rt], ot[:, :, :rt])

    nc.finalize()
    return nc


_NC_CACHE = {}
_last_in_maps = None


def _get_nc(mode):
    if mode not in _NC_CACHE:
        _NC_CACHE[mode] = _build_nc(mode)
    return _NC_CACHE[mode]


def kernel(x, Wb, WA, WB):
    x = np.asarray(x, dtype=np.float32)
    Wb = np.asarray(Wb, dtype=np.float32)
    WA = np.asarray(WA, dtype=np.float32)
    WB = np.asarray(WB, dtype=np.float32)

    # fold LoRA into the base weight (float64 for the tiny weight math)
    pw_base = 1.0 / np.sqrt(np.float64(MUL))
    pw_B = 1.0 / np.sqrt(np.float64(RANK))
    W_eff = (
        pw_base * Wb.astype(np.float64)
        + SCALING * pw_base * pw_B * (WA.astype(np.float64) @ WB.astype(np.float64))
    ).astype(np.float32)

    wpk = _pack_weights(W_eff)
    three_pass = MODE in ("f32r3", "f16x3")
    if MODE == "f16x1":
        wh = wpk.astype(np.float16)
        wl = None
    elif MODE == "f16x3":
        wh = wpk.astype(np.float16)
        wl = (wpk - wh.astype(np.float32)).astype(np.float16)
    elif three_pass:
        wh = (wpk.view(np.uint32) & _MASK11).view(np.float32)
        wl = wpk - wh
    else:
        wh = wpk
        wl = None

    # per-core transposed, padded inputs
    in_maps = []
    for i in range(NCORES):
        xt = np.zeros((FPAD, ROWS), dtype=np.float32)
        xt[:FEAT] = x[i * ROWS : (i + 1) * ROWS].T
        if MODE == "f16x1":
            x1p = xt[:FEAT].astype(np.float16)  # [1728, ROWS]
            chunks = x1p[: NSEC_FULL * 128].reshape(NSEC_FULL, 128, ROWS)
            x1a = np.empty((128, NSEC_FULL * ROWS), dtype=np.float16)
            for ti, (r0, rt) in enumerate(_row_tiles_f16x1()):
                x1a[:, NSEC_FULL * r0 : NSEC_FULL * (r0 + rt)] = (
                    chunks[:, :, r0 : r0 + rt]
                    .transpose(1, 0, 2)
                    .reshape(128, NSEC_FULL * rt)
                )
            x1b = np.ascontiguousarray(x1p[NSEC_FULL * 128 :])  # [64, ROWS]
            m = {"x1a": x1a, "x1b": x1b, "wh": wh}
        elif MODE == "f16x3":
            x1p = xt.astype(np.float16)
            x2p = (xt - x1p.astype(np.float32)).astype(np.float16)
            tiles = _row_tiles(RF16)
            x1 = np.zeros((len(tiles), 128, NSEC * RF16), dtype=np.float16)
            x2 = np.zeros_like(x1)
            for ti, (r0, rt) in enumerate(tiles):
                a = x1p[:, r0 : r0 + rt].reshape(NSEC, 128, rt)
                b = x2p[:, r0 : r0 + rt].reshape(NSEC, 128, rt)
                v1 = x1[ti].reshape(128, NSEC, RF16)
                v2 = x2[ti].reshape(128, NSEC, RF16)
                v1[:, :, :rt] = a.transpose(1, 0, 2)
                v2[:, :, :rt] = b.transpose(1, 0, 2)
            m = {"x1": x1, "x2": x2, "wh": wh, "wl": wl}
        else:
            m = {"xt": xt, "wh": wh}
            if three_pass:
                m["wl"] = wl
        in_maps.append(m)

    global _last_in_maps
    _last_in_maps = in_maps
    nc = _get_nc(MODE)
    res = run_bass_kernel_spmd(nc, in_maps, core_ids=list(range(NCORES)))

    out = np.empty((N_NODES, FEAT), dtype=np.float32)
    for i in range(NCORES):
        if MODE == "f16x1":
            ota = res.results[i]["ota"]
            otb = res.results[i]["otb"]
            oc = out[i * ROWS : (i + 1) * ROWS]
            for ti, (r0, rt) in enumerate(_row_tiles_f16x1()):
                blk = ota[:, NSEC_FULL * r0 : NSEC_FULL * (r0 + rt)].reshape(
                    128, NSEC_FULL, rt
                )
                oc[r0 : r0 + rt, : NSEC_FULL * 128] = (
                    blk.transpose(2, 1, 0)
                    .reshape(rt, NSEC_FULL * 128)
                    .astype(np.float32)
                )
            oc[:, NSEC_FULL * 128 :] = otb.T.astype(np.float32)
        else:
            out[i * ROWS : (i + 1) * ROWS] = res.results[i]["ot"][:FEAT].T.astype(
                np.float32
            )
    return out



# revision 58
# speedup vs baseline: 1.0006x; 1.0006x over previous
"""TRN2 Bass kernel for nn_LoRACuetLinear (equivariant LoRA linear).

Math: for each irrep block j (9 blocks of 192 features; block j uses irrep
k(j) in {0,1,2}), out_seg = seg @ W_eff[k] where
  W_eff[k] = pw_base * Wb[k] + SCALING * pw_base * pw_B * (WA[k] @ WB[k])
(the LoRA branch folds exactly into the base weight since everything is
linear).

Device strategy (8 cores, data-parallel over nodes):
  - Host transposes x to x_T [features, rows] per core so the contraction
    dim (mul/feature) lies on SBUF partitions; the device then runs
    weights-stationary matmuls out_T = W^T x_T with the moving dim = rows.
  - Default mode "f16x1" (~5e-4 absmax rel, vs the 2e-2 gate): single fp16
    plane of x, single matmul pass, fp16 output.  Relative to the fp32-
    accurate "f16x3" this cuts HBM traffic 94.8 -> 44.3 MB/core and PE work
    3x (measured 351us -> ~143-147us):
      * Tile-major fully-contiguous dram layouts ([128, 13*ROWS] +
        [64, ROWS] per direction, exact-size SBUF tiles per row-tile size):
        every DMA merges into one fat run per partition.  Sliced/strided
        transfers degrade to small segments and lose ~25% DMA rate.
      * Input DMAs dispatch on the Sync queue, output DMAs on the Scalar
        (Activation) HWDGE queue: out-dispatch waits (copies done) never
        head-of-line-block input prefetch.
      * ALL psum->sbuf cast-copies on Scalar (no Vector/GpSimd split): a
        single self-ordered queue [copies_i, out_i, copies_i+1 ...].  Every
        engine-splitting variant measured slower (psum ring banks recycle
        in-order; bursty cross-engine copy assignment convoys the PE).
      * Row-tile schedule (256, 1024x5, 618, 256): small first tile so the
        PE starts ~6us earlier, small last tile so the final unoverlapped
        output drain is short, and as FEW tiles as possible -- every tile
        pays a full 32-LDWEIGHTS sweep regardless of rows (consolidating
        the tail from [512,256,106] to [618,256] measured ~5us faster).
      * Steady state is HBM-bound: 16 DMA engines ~385 GB/s sustained with
        ~15us fixed framework preamble/teardown.
  - Weights (LoRA exactly folded into the base, fp16) are packed per
    128-row output section into a block-diagonal [128, 32*128] layout so
    every matmul has M=128 at psum partition base 0.
  - Fallback modes kept for experiments: "f16x3" (fp32-accurate 3-pass),
    "f32r3" (float32r 3-pass with on-device DVE split) and "f32r1"
    (single-pass float32r, ~1e-4 rel).
"""

import sys

sys.path.insert(0, "/opt/trn_rl_repo")

import os
import numpy as np

import concourse.bass as bass
import concourse.tile as tile
from concourse import bacc, mybir
from concourse.bass_utils import run_bass_kernel_spmd
# ---- problem constants (hardcoded per contract) ----
MUL = 192
DIMS = (1, 3, 5)
RANK = 8
SCALING = 2.0
N_NODES = 50000
FEAT = MUL * sum(DIMS)  # 1728
NCORES = 8
ROWS = N_NODES // NCORES  # 6250
FPAD = 1792  # 14 * 128
NSEC = FPAD // 128  # 14
R = 352  # row-tile (moving dim); 6250 = 17*352 + 266 (all tiles >= 256)
RF16 = 512  # row-tile for the f16 path (smaller SBUF tiles allow 512)
RT1 = 1024  # row-tile for the single-pass f16 path (two 512 psum halves)
PSUM_N = 512  # max fp32 cols per psum bank
MODE = os.environ.get("LORA_KERNEL_MODE", "f16x1")  # f16x1 | f16x3 | f32r3 | f32r1
BLK_IRREP = [0] + [1] * 3 + [2] * 5

_MASK11 = np.uint32(0xFFFFF000)  # keep sign+exp+11 mantissa bits


def _section_mms():
    """Enumerate matmuls as (section, chunk, r0, r1, windex).

    Section s covers padded output rows [128s, 128s+128); chunk c covers
    padded input rows [128c, 128c+128).  (s, c) participates iff the
    block-diagonal weight has support there; r0:r1 is the nonzero input-row
    range within the chunk (always base 0 or 64, size 64 or 128).
    """
    sup = np.zeros((FPAD, FPAD), dtype=bool)
    for j in range(sum(DIMS)):
        sup[192 * j : 192 * j + 192, 192 * j : 192 * j + 192] = True
    mms = []
    wi = 0
    for s in range(NSEC):
        for c in range(NSEC):
            sl = sup[128 * c : 128 * c + 128, 128 * s : 128 * s + 128]
            nz = np.nonzero(sl.any(axis=1))[0]
            if len(nz) == 0:
                continue
            r0 = (int(nz[0]) // 64) * 64
            r1 = ((int(nz[-1]) + 64) // 64) * 64
            mms.append((s, c, r0, r1, wi))
            wi += 1
    return mms


_MMS = _section_mms()
NW = len(_MMS)  # 32 packed weight slots of [128, 128]


def _pack_weights(W_eff):
    """Build the packed per-section weight [128, NW*128] from W_eff [3,192,192]."""
    W_big = np.zeros((FPAD, FPAD), dtype=np.float32)
    for j, k in enumerate(BLK_IRREP):
        W_big[192 * j : 192 * j + 192, 192 * j : 192 * j + 192] = W_eff[k]
    wpk = np.zeros((128, NW * 128), dtype=np.float32)
    for s, c, r0, r1, wi in _MMS:
        wpk[:, wi * 128 : (wi + 1) * 128] = W_big[
            128 * c : 128 * c + 128, 128 * s : 128 * s + 128
        ]
    return wpk


def _row_tiles(r):
    tiles = []
    r0 = 0
    while r0 < ROWS:
        tiles.append((r0, min(r, ROWS - r0)))
        r0 += r
    return tiles


# f16x1 tile schedule: small first tile so the PE starts after a 0.9MB DMA
# instead of 3.7MB, small last tile so the final (unoverlappable) output DMA
# is short.  Sums to ROWS.
F16X1_SIZES = (256, 1024, 1024, 1024, 1024, 1024, 618, 256)
assert sum(F16X1_SIZES) == ROWS
NSEC_FULL = NSEC - 1  # 13 full 128-partition chunks; chunk 13 has 64 rows
# sbuf ring depth per tile size (1024 is the steady-state size; the odd
# sizes appear at the ramp/tail and never overlap themselves)
F16X1_BUFS = {1024: 3, 618: 1, 512: 1, 256: 1, 106: 1}


def _row_tiles_f16x1():
    tiles = []
    r0 = 0
    for sz in F16X1_SIZES:
        tiles.append((r0, sz))
        r0 += sz
    return tiles


def _build_nc_f16x1():
    """Single-pass fp16 kernel with fp16 output (~4.5e-4 absmax rel).

    The 2e-2 correctness gate leaves ~40x headroom over fp16 quantization
    noise, so ship one fp16 plane of x (half the input bytes of the fp32-
    accurate path), run one matmul pass (1/3 the PE work), and return the
    output as fp16 (half the output bytes).  psum->sbuf cast-copies
    alternate between the Scalar and Vector engines so neither serializes
    against the ~139us of DMA this leaves.
    """
    f32 = mybir.dt.float32
    f16 = mybir.dt.float16

    nc = bacc.Bacc("TRN2", target_bir_lowering=False, debug=False)
    # Tile-major contiguous dram layout: every transfer is one large
    # contiguous segment per partition (no 512B/2KB striding), for any tile
    # size.  Chunk 13 only has 64 real partitions (FEAT = 13.5 * 128), so it
    # lives in its own [64, ROWS] params and the zero half is never moved.
    x1a_in = nc.declare_dram_parameter(
        "x1a", [128, NSEC_FULL * ROWS], f16, isOutput=False
    )
    x1b_in = nc.declare_dram_parameter("x1b", [64, ROWS], f16, isOutput=False)
    wh_in = nc.declare_dram_parameter("wh", [128, NW * 128], f16, isOutput=False)
    ota_out = nc.declare_dram_parameter(
        "ota", [128, NSEC_FULL * ROWS], f16, isOutput=True
    )
    otb_out = nc.declare_dram_parameter("otb", [64, ROWS], f16, isOutput=True)

    sec_list = [[m for m in _MMS if m[0] == s] for s in range(NSEC)]

    with tile.TileContext(nc) as tc:
        with (
            tc.tile_pool(name="wp", bufs=1) as wp,
            tc.tile_pool(name="hp", bufs=1) as hp,
            tc.tile_pool(name="op", bufs=1) as op,
            tc.tile_pool(name="ps", bufs=8, space="PSUM") as ps,
        ):
            wh = wp.tile([128, NW * 128], f16, tag="wh")

            for wave in [[t] for t in enumerate(_row_tiles_f16x1())]:
                xhs, ots, odsts = [], [], []
                for ti, (r0, rt) in wave:
                    # exact-size tiles per rt: a fully dense destination lets
                    # the DMA merge each partition's data into one contiguous
                    # run (sliced tiles degrade to rt*2-byte segments)
                    xh = hp.tile(
                        [128, NSEC, rt], f16, tag=f"xh{rt}", name=f"xh{rt}",
                        bufs=F16X1_BUFS[rt],
                    )
                    xsrc = x1a_in.ap()[
                        :, NSEC_FULL * r0 : NSEC_FULL * (r0 + rt)
                    ].rearrange("p (c r) -> p c r", c=NSEC_FULL)
                    nc.sync.dma_start(xh[:, :NSEC_FULL, :], xsrc)
                    nc.sync.dma_start(
                        xh[0:64, NSEC_FULL, :], x1b_in.ap()[:, r0 : r0 + rt]
                    )
                    if ti == 0:
                        # weights dispatched after tile 0's input: engine
                        # queues drain FIFO, so the small first input lands
                        # first.  Head reordering dead ends (measured):
                        # splitting wh (+12us, its tail queued behind tile
                        # 1's input); sending tile 0's first chunks ahead of
                        # wh (first matmul starts 2.8us earlier but tile 0
                        # then just idles for tile 1's FIFO-bounded arrival
                        # -- net zero).
                        nc.sync.dma_start(wh[:], wh_in[:])
                    ot = op.tile(
                        [128, NSEC, rt], f16, tag=f"ot{rt}", name=f"ot{rt}",
                        bufs=min(2, F16X1_BUFS[rt]),
                    )
                    xhs.append(xh)
                    ots.append(ot)
                    odsts.append(
                        ota_out.ap()[
                            :, NSEC_FULL * r0 : NSEC_FULL * (r0 + rt)
                        ].rearrange("p (c r) -> p c r", c=NSEC_FULL)
                    )

                for s in range(NSEC):
                    # (tile, half) pairs processed weight-major so matmuls
                    # sharing a stationary slice are adjacent and walrus
                    # elides the LDWEIGHTS reloads
                    parts = [
                        (w, h0, min(PSUM_N, rt - h0))
                        for w, (ti, (r0, rt)) in enumerate(wave)
                        for h0 in range(0, rt, PSUM_N)
                    ]
                    psums = [
                        ps.tile([128, PSUM_N], f32, tag="ps", name=f"ps{pi}")
                        for pi in range(len(parts))
                    ]
                    nmm = len(sec_list[s])
                    for i, (_, c, k0, k1, wi) in enumerate(sec_list[s]):
                        for pi, (w, h0, hn) in enumerate(parts):
                            nc.tensor.matmul(
                                psums[pi][:, :hn],
                                wh[k0:k1, wi * 128 : (wi + 1) * 128],
                                xhs[w][k0:k1, c, h0 : h0 + hn],
                                start=(i == 0),
                                stop=(i == nmm - 1),
                            )
                    mp = 64 if s == NSEC - 1 else 128  # real out rows in section
                    # all copies on Scalar: one self-ordered queue (copies +
                    # out-DMA dispatches).  Measured dead ends: every
                    # Vector-copy split was 10-27us slower (DVE psum reads
                    # appear to contend with PE psum writes), and GpSimd
                    # psum-source copies fail to compile in neuronxcc.
                    for pi, (w, h0, hn) in enumerate(parts):
                        nc.scalar.copy(
                            ots[w][:mp, s, h0 : h0 + hn], psums[pi][:mp, :hn]
                        )
                # output DMAs dispatch whole-tile on the Scalar (Activation)
                # HWDGE queue so their copy-completion waits never head-of-
                # line-block the input prefetches on the Sync queue.
                # Measured dead ends: outputs on Sync (~+5us, head-of-line),
                # and mid-tile split outputs on either queue (+4..+10us --
                # the extra dispatches and partial-tile transfers cost more
                # than the stream smoothing buys).
                for w, (ti, (r0, rt)) in enumerate(wave):
                    nc.scalar.dma_start(
                        odsts[w][:, :NSEC_FULL, :], ots[w][:, :NSEC_FULL, :]
                    )
                    nc.scalar.dma_start(
                        otb_out.ap()[:, r0 : r0 + rt],
                        ots[w][0:64, NSEC_FULL, :],
                    )

    nc.finalize()
    return nc


def _build_nc(mode):
    if mode == "f16x1":
        return _build_nc_f16x1()
    fr = mybir.dt.float32r
    f32 = mybir.dt.float32
    f16 = mybir.dt.float16
    f16_mode = mode == "f16x3"
    three_pass = mode in ("f32r3", "f16x3")
    wdt = f16 if f16_mode else fr
    r_tile = RF16 if f16_mode else R

    nc = bacc.Bacc("TRN2", target_bir_lowering=False, debug=False)
    if f16_mode:
        # host pre-splits x into two fp16 planes (x = x1 + x2 to 22 bits),
        # pre-tiled as [rowtile, partition, chunk*R] so each partition's
        # per-rowtile data is one contiguous segment for the DMA
        nt = len(_row_tiles(r_tile))
        x1_in = nc.declare_dram_parameter(
            "x1", [nt, 128, NSEC * r_tile], f16, isOutput=False
        )
        x2_in = nc.declare_dram_parameter(
            "x2", [nt, 128, NSEC * r_tile], f16, isOutput=False
        )
    else:
        xdt_dram = f32 if three_pass else fr
        xt_in = nc.declare_dram_parameter("xt", [FPAD, ROWS], xdt_dram, isOutput=False)
        xt_src = xt_in.ap().rearrange("(c p) r -> p c r", p=128)
    wh_in = nc.declare_dram_parameter("wh", [128, NW * 128], wdt, isOutput=False)
    if three_pass:
        wl_in = nc.declare_dram_parameter("wl", [128, NW * 128], wdt, isOutput=False)
    ot_out = nc.declare_dram_parameter("ot", [FPAD, ROWS], f32, isOutput=True)

    ot_dst = ot_out.ap().rearrange("(c p) r -> p c r", p=128)

    sec_list = [[m for m in _MMS if m[0] == s] for s in range(NSEC)]

    xbufs = 3 if f16_mode else 2
    with tile.TileContext(nc) as tc:
        with (
            tc.tile_pool(name="wp", bufs=1) as wp,
            tc.tile_pool(name="xp", bufs=2) as xp,
            tc.tile_pool(name="hp", bufs=xbufs) as hp,
            tc.tile_pool(name="lp", bufs=xbufs) as lp,
            tc.tile_pool(name="op", bufs=2) as op,
            tc.tile_pool(name="ps", bufs=6, space="PSUM") as ps,
        ):
            wh = wp.tile([128, NW * 128], wdt, tag="wh")
            nc.sync.dma_start(wh[:], wh_in[:])
            if three_pass:
                wl = wp.tile([128, NW * 128], wdt, tag="wl")
                nc.sync.dma_start(wl[:], wl_in[:])

            for ti, (r0, rt) in enumerate(_row_tiles(r_tile)):
                if f16_mode:
                    xh = hp.tile([128, NSEC, r_tile], f16, tag="xh")
                    xl = lp.tile([128, NSEC, r_tile], f16, tag="xl")
                    nc.sync.dma_start(
                        xh[:], x1_in[ti].rearrange("p (c r) -> p c r", c=NSEC)
                    )
                    nc.sync.dma_start(
                        xl[:], x2_in[ti].rearrange("p (c r) -> p c r", c=NSEC)
                    )
                    passes = [(xh, wh), (xl, wh), (xh, wl)]
                elif three_pass:
                    # X1 = rn11(X), X2 = rn11(X - X1).  The raw X tile must be
                    # a genuine float32 memloc: walrus rounds float32r-memloc
                    # inputs on read, so an in-place split would cancel to 0.
                    # Rounding happens on the DVE cast writes.
                    x = xp.tile([128, NSEC, r_tile], f32, tag="x")
                    nc.sync.dma_start(x[:, :, :rt], xt_src[:, :, r0 : r0 + rt])
                    xh = hp.tile([128, NSEC, r_tile], wdt, tag="xh")
                    xl = lp.tile([128, NSEC, r_tile], wdt, tag="xl")
                    nc.vector.tensor_copy(xh[:, :, :rt], x[:, :, :rt])
                    nc.vector.tensor_sub(xl[:, :, :rt], x[:, :, :rt], xh[:, :, :rt])
                    passes = [(xh, wh), (xl, wh), (xh, wl)]
                else:
                    x = xp.tile([128, NSEC, r_tile], fr, tag="x")
                    nc.sync.dma_start(x[:, :, :rt], xt_src[:, :, r0 : r0 + rt])
                    passes = [(x, wh)]

                ot = op.tile([128, NSEC, r_tile], f32, tag="ot")
                for s in range(NSEC):
                    psum = ps.tile([128, r_tile], f32, tag="ps")
                    # order so matmuls sharing a stationary slice are
                    # adjacent (lets walrus ldw-opt elide reloads)
                    if len(passes) == 3:
                        (xa, wa), (xb, _), (_, wc) = passes
                        seq = [
                            (x, w, c, k0, k1, wi)
                            for _, c, k0, k1, wi in sec_list[s]
                            for x, w in ((xa, wa), (xb, wa))
                        ] + [
                            (xa, wc, c, k0, k1, wi)
                            for _, c, k0, k1, wi in sec_list[s]
                        ]
                    else:
                        seq = [
                            (x, w, c, k0, k1, wi)
                            for x, w in passes
                            for _, c, k0, k1, wi in sec_list[s]
                        ]
                    for i, (xsrc, wsrc, c, k0, k1, wi) in enumerate(seq):
                        nc.tensor.matmul(
                            psum[:, :rt],
                            wsrc[k0:k1, wi * 128 : (wi + 1) * 128],
                            xsrc[k0:k1, c, :rt],
                            start=(i == 0),
                            stop=(i == len(seq) - 1),
                        )
                    nc.scalar.copy(ot[:, s, :rt], psum[:, :rt])
                nc.sync.dma_start(ot_dst[:, :, r0 : r0 + rt], ot[:, :, :rt])

    nc.finalize()
    return nc


_NC_CACHE = {}
_last_in_maps = None


def _get_nc(mode):
    if mode not in _NC_CACHE:
        _NC_CACHE[mode] = _build_nc(mode)
    return _NC_CACHE[mode]


def kernel(x, Wb, WA, WB):
    x = np.asarray(x, dtype=np.float32)
    Wb = np.asarray(Wb, dtype=np.float32)
    WA = np.asarray(WA, dtype=np.float32)
    WB = np.asarray(WB, dtype=np.float32)

    # fold LoRA into the base weight (float64 for the tiny weight math)
    pw_base = 1.0 / np.sqrt(np.float64(MUL))
    pw_B = 1.0 / np.sqrt(np.float64(RANK))
    W_eff = (
        pw_base * Wb.astype(np.float64)
        + SCALING * pw_base * pw_B * (WA.astype(np.float64) @ WB.astype(np.float64))
    ).astype(np.float32)

    wpk = _pack_weights(W_eff)
    three_pass = MODE in ("f32r3", "f16x3")
    if MODE == "f16x1":
        wh = wpk.astype(np.float16)
        wl = None
    elif MODE == "f16x3":
        wh = wpk.astype(np.float16)
        wl = (wpk - wh.astype(np.float32)).astype(np.float16)
    elif three_pass:
        wh = (wpk.view(np.uint32) & _MASK11).view(np.float32)
        wl = wpk - wh
    else:
        wh = wpk
        wl = None

    # per-core transposed, padded inputs
    in_maps = []
    for i in range(NCORES):
        xt = np.zeros((FPAD, ROWS), dtype=np.float32)
        xt[:FEAT] = x[i * ROWS : (i + 1) * ROWS].T
        if MODE == "f16x1":
            x1p = xt[:FEAT].astype(np.float16)  # [1728, ROWS]
            chunks = x1p[: NSEC_FULL * 128].reshape(NSEC_FULL, 128, ROWS)
            x1a = np.empty((128, NSEC_FULL * ROWS), dtype=np.float16)
            for ti, (r0, rt) in enumerate(_row_tiles_f16x1()):
                x1a[:, NSEC_FULL * r0 : NSEC_FULL * (r0 + rt)] = (
                    chunks[:, :, r0 : r0 + rt]
                    .transpose(1, 0, 2)
                    .reshape(128, NSEC_FULL * rt)
                )
            x1b = np.ascontiguousarray(x1p[NSEC_FULL * 128 :])  # [64, ROWS]
            m = {"x1a": x1a, "x1b": x1b, "wh": wh}
        elif MODE == "f16x3":
            x1p = xt.astype(np.float16)
            x2p = (xt - x1p.astype(np.float32)).astype(np.float16)
            tiles = _row_tiles(RF16)
            x1 = np.zeros((len(tiles), 128, NSEC * RF16), dtype=np.float16)
            x2 = np.zeros_like(x1)
            for ti, (r0, rt) in enumerate(tiles):
                a = x1p[:, r0 : r0 + rt].reshape(NSEC, 128, rt)
                b = x2p[:, r0 : r0 + rt].reshape(NSEC, 128, rt)
                v1 = x1[ti].reshape(128, NSEC, RF16)
                v2 = x2[ti].reshape(128, NSEC, RF16)
                v1[:, :, :rt] = a.transpose(1, 0, 2)
                v2[:, :, :rt] = b.transpose(1, 0, 2)
            m = {"x1": x1, "x2": x2, "wh": wh, "wl": wl}
        else:
            m = {"xt": xt, "wh": wh}
            if three_pass:
                m["wl"] = wl
        in_maps.append(m)

    global _last_in_maps
    _last_in_maps = in_maps
    nc = _get_nc(MODE)
    res = run_bass_kernel_spmd(nc, in_maps, core_ids=list(range(NCORES)))

    out = np.empty((N_NODES, FEAT), dtype=np.float32)
    for i in range(NCORES):
        if MODE == "f16x1":
            ota = res.results[i]["ota"]
            otb = res.results[i]["otb"]
            oc = out[i * ROWS : (i + 1) * ROWS]
            for ti, (r0, rt) in enumerate(_row_tiles_f16x1()):
                blk = ota[:, NSEC_FULL * r0 : NSEC_FULL * (r0 + rt)].reshape(
                    128, NSEC_FULL, rt
                )
                oc[r0 : r0 + rt, : NSEC_FULL * 128] = (
                    blk.transpose(2, 1, 0)
                    .reshape(rt, NSEC_FULL * 128)
                    .astype(np.float32)
                )
            oc[:, NSEC_FULL * 128 :] = otb.T.astype(np.float32)
        else:
            out[i * ROWS : (i + 1) * ROWS] = res.results[i]["ot"][:FEAT].T.astype(
                np.float32
            )
    return out

